# revision 12
# baseline (speedup 1.0000x reference)
"""Multi-head self-attention (B=4, T=2048, C=1024, H=16, D=64) on 8 NeuronCores.

Sharding: tensor-parallel over heads (Megatron): each core owns 2 heads.
Wq/Wk/Wv column-sharded, Wo row-sharded; host sums the 8 partial outputs.

Device layout is fully "transposed" (features on partitions, tokens on the
free dim) so that softmax runs over the PSUM free dim and the PV matmul needs
no attention-matrix transpose. The softmax denominator comes from a ones
column appended to V (M=65 stationary), landing in row 64 of the PV PSUM.

The PE executes its stream in order, so projection/output-projection matmuls
of adjacent batches are interleaved into the attention j-loop (which is
paced by the Scalar engine's exp) to keep both engines near 100% busy.
"""

import numpy as np

import concourse.bass as bass
import concourse.tile as tile
from concourse import bacc, mybir
from concourse.bass_utils import run_bass_kernel_spmd

B, T, C, H, D = 4, 2048, 1024, 16, 64
NCORES = 8
HPC = H // NCORES          # heads per core = 2
F = HPC * D                # per-core feature width = 128
TT = B * T                 # total tokens = 8192

FP32 = mybir.dt.float32
MM_DT = mybir.dt.bfloat16  # matmul compute dtype

TILE_K = 128               # contraction tile
TILE_N = 512               # moving free dim per matmul
NK_C = C // TILE_K         # 8 k-tiles over channels
NT4 = T // TILE_N          # 4 token chunks per batch
NJ = T // TILE_K           # 16 key tiles per batch


def build_kernel_body(tc):
    nc = tc.nc
    Exp = mybir.ActivationFunctionType.Exp

    xT = nc.dram_tensor("xT", [C, TT], MM_DT, kind="ExternalInput").ap()
    wq = nc.dram_tensor("wq", [C, F], MM_DT, kind="ExternalInput").ap()
    wk = nc.dram_tensor("wk", [C, F], MM_DT, kind="ExternalInput").ap()
    wv = nc.dram_tensor("wv", [C, F], MM_DT, kind="ExternalInput").ap()
    wo = nc.dram_tensor("wo", [F, C], MM_DT, kind="ExternalInput").ap()
    bqv = nc.dram_tensor("bq", [F], FP32, kind="ExternalInput").ap()
    bkv = nc.dram_tensor("bk", [F], FP32, kind="ExternalInput").ap()
    outT = nc.dram_tensor("outT", [C, TT], FP32, kind="ExternalOutput").ap()

    import contextlib
    ctx = contextlib.ExitStack()
    with ctx:
        consts = ctx.enter_context(tc.tile_pool(name="consts", bufs=1))
        xpool = ctx.enter_context(tc.tile_pool(name="xt", bufs=12))
        bigs = ctx.enter_context(tc.tile_pool(name="bigs", bufs=2))
        epool = ctx.enter_context(tc.tile_pool(name="expp", bufs=3))
        small = ctx.enter_context(tc.tile_pool(name="small", bufs=2))
        vstage = ctx.enter_context(tc.tile_pool(name="vstage", bufs=2))
        ps_qk = ctx.enter_context(tc.tile_pool(name="ps_qk", bufs=2, space="PSUM"))
        ps_pv = ctx.enter_context(tc.tile_pool(name="ps_pv", bufs=1, space="PSUM"))
        ps_aux = ctx.enter_context(tc.tile_pool(name="ps_aux", bufs=2, space="PSUM"))

        # ---- constants ----
        wq_sb = consts.tile([TILE_K, C], MM_DT)  # c-tile k at [:, k*F:(k+1)*F]
        nc.sync.dma_start(
            wq_sb[:].rearrange("p (k f) -> p k f", k=NK_C),
            wq.rearrange("(k p) f -> p k f", p=TILE_K))
        wk_sb = consts.tile([TILE_K, C], MM_DT)
        nc.sync.dma_start(
            wk_sb[:].rearrange("p (k f) -> p k f", k=NK_C),
            wk.rearrange("(k p) f -> p k f", p=TILE_K))
        wv_sb = consts.tile([TILE_K, C], MM_DT)
        nc.sync.dma_start(
            wv_sb[:].rearrange("p (k f) -> p k f", k=NK_C),
            wv.rearrange("(k p) f -> p k f", p=TILE_K))
        wo_sb = consts.tile([F, C], MM_DT)
        nc.sync.dma_start(wo_sb[:], wo)
        bq_sb = consts.tile([F, 1], FP32)
        nc.sync.dma_start(bq_sb[:], bqv.rearrange("(p one) -> p one", one=1))
        bk_sb = consts.tile([F, 1], FP32)
        nc.sync.dma_start(bk_sb[:], bkv.rearrange("(p one) -> p one", one=1))
        ident = consts.tile([128, 128], FP32)
        from concourse.masks import make_identity
        make_identity(nc, ident[:])
        ones32 = consts.tile([128, NJ * HPC], FP32)
        nc.gpsimd.memset(ones32[:], 1.0)

        tiles = {}  # per-batch SBUF tiles

        def alloc_proj_tiles(b):
            qT = bigs.tile([F, T], MM_DT, tag="qT", name=f"qT{b}")
            kT = bigs.tile([F, T], MM_DT, tag="kT", name=f"kT{b}")
            v1 = bigs.tile([128, NJ * HPC, D + 1], MM_DT, tag="v1",
                           name=f"v1_{b}")
            nc.vector.tensor_copy(
                v1[:, :, D : D + 1],
                ones32[:].rearrange("p (a b) -> p a b", b=1),
            )
            tiles[b] = {"qT": qT, "kT": kT, "v1": v1}

        def gen_proj(b):
            """Generator: projections for batch b; yields after each PE op."""
            alloc_proj_tiles(b)
            tl = tiles[b]
            t0 = b * T
            for t4 in range(NT4):
                xts = []
                for kk in range(NK_C):
                    xt = xpool.tile([TILE_K, TILE_N], MM_DT, tag="xt")
                    nc.sync.dma_start(
                        xt[:],
                        xT[kk * TILE_K : (kk + 1) * TILE_K,
                           t0 + t4 * TILE_N : t0 + (t4 + 1) * TILE_N],
                    )
                    xts.append(xt)
                for which, w_sb in (("q", wq_sb), ("k", wk_sb), ("v", wv_sb)):
                    acc = ps_aux.tile([128, TILE_N], FP32, tag="aux")
                    for kk in range(NK_C):
                        nc.tensor.matmul(
                            acc[:], w_sb[:, kk * F : (kk + 1) * F], xts[kk][:],
                            start=(kk == 0), stop=(kk == NK_C - 1),
                        )
                        yield
                    if which == "q":
                        nc.vector.tensor_scalar_add(
                            tl["qT"][:, t4 * TILE_N : (t4 + 1) * TILE_N],
                            acc[:], bq_sb[:])
                    elif which == "k":
                        nc.vector.tensor_scalar_add(
                            tl["kT"][:, t4 * TILE_N : (t4 + 1) * TILE_N],
                            acc[:], bk_sb[:])
                    else:
                        vt_sb = vstage.tile([128, TILE_N], FP32, tag="vt")
                        nc.vector.tensor_copy(vt_sb[:], acc[:])
                        for tt in range(TILE_N // 128):
                            j_idx = t4 * (TILE_N // 128) + tt
                            ptr = ps_aux.tile([128, TILE_N], FP32, tag="aux")
                            nc.tensor.transpose(
                                ptr[:, 0:128],
                                vt_sb[:, tt * 128 : (tt + 1) * 128], ident[:],
                            )
                            # both heads' v columns in one strided copy
                            nc.vector.tensor_copy(
                                tl["v1"][:, j_idx * HPC : (j_idx + 1) * HPC, 0:D],
                                ptr[:, 0:128].rearrange("p (h d) -> p h d", h=HPC),
                            )
                            yield

        def gen_wo(b):
            """Generator: output projection for batch b; yields per PE op."""
            t0 = b * T
            ctxT = tiles[b]["ctxT"]
            for t4 in range(NT4):
                for o in range(C // 128):
                    po = ps_aux.tile([128, TILE_N], FP32, tag="aux")
                    nc.tensor.matmul(
                        po[:], wo_sb[:, o * 128 : (o + 1) * 128],
                        ctxT[:, t4 * TILE_N : (t4 + 1) * TILE_N],
                        start=True, stop=True,
                    )
                    osb = vstage.tile([128, TILE_N], FP32, tag="osb", bufs=3)
                    nc.vector.tensor_copy(osb[:], po[:])
                    nc.sync.dma_start(
                        outT[o * 128 : (o + 1) * 128,
                             t0 + t4 * TILE_N : t0 + (t4 + 1) * TILE_N],
                        osb[:],
                    )
                    yield

        # prologue: projections for batch 0 (PE-only ramp)
        for _ in gen_proj(0):
            pass

        for b in range(B):
            tl = tiles[b]
            ctxT = bigs.tile([F, T], MM_DT, tag="ctxT", name=f"ctxT{b}")
            tl["ctxT"] = ctxT
            qT, kT, v1 = tl["qT"], tl["kT"], tl["v1"]

            fillers = []
            if b + 1 < B:
                fillers.append(gen_proj(b + 1))
            if b > 0:
                fillers.append(gen_wo(b - 1))
            n_fill = (112 if b + 1 < B else 0) + (32 if b > 0 else 0)
            pulled = 0

            def pull(target):
                nonlocal pulled
                while pulled < target and fillers:
                    try:
                        next(fillers[0])
                        pulled += 1
                    except StopIteration:
                        fillers.pop(0)

            for i4 in range(NT4):
                isl = slice(i4 * TILE_N, (i4 + 1) * TILE_N)
                pv0 = ps_pv.tile([128, TILE_N], FP32, tag="pv0")
                pv1 = ps_pv.tile([128, TILE_N], FP32, tag="pv1")
                expts = [None] * NJ
                for j in range(NJ):
                    jsl = slice(j * TILE_K, (j + 1) * TILE_K)
                    qk = ps_qk.tile([128, 2 * TILE_N], FP32, tag="qk")
                    # heads in distinct PE row-groups -> run concurrently
                    nc.tensor.matmul(qk[:, 0:TILE_N], kT[0:D, jsl],
                                     qT[0:D, isl], start=True, stop=True)
                    nc.tensor.matmul(qk[:, TILE_N : 2 * TILE_N],
                                     kT[D : 2 * D, jsl], qT[D : 2 * D, isl],
                                     start=True, stop=True)
                    expt = epool.tile([128, 2 * TILE_N], MM_DT, tag="expt")
                    nc.scalar.activation(expt[:], qk[:], Exp)
                    expts[j] = expt
                    # keep the in-order PE stream fed while ACT runs exp
                    slot = i4 * NJ + j + 1
                    pull((slot * n_fill + NT4 * NJ - 1) // (NT4 * NJ))
                    if j > 0:
                        e = expts[j - 1]
                        nc.tensor.matmul(
                            pv0[0 : D + 1, :], v1[:, (j - 1) * HPC + 0, :],
                            e[:, 0:TILE_N], start=(j == 1), stop=False)
                        nc.tensor.matmul(
                            pv1[0 : D + 1, :], v1[:, (j - 1) * HPC + 1, :],
                            e[:, TILE_N : 2 * TILE_N], start=(j == 1),
                            stop=False)
                e = expts[NJ - 1]
                nc.tensor.matmul(pv0[0 : D + 1, :], v1[:, (NJ - 1) * HPC + 0, :],
                                 e[:, 0:TILE_N], start=False, stop=True)
                nc.tensor.matmul(pv1[0 : D + 1, :], v1[:, (NJ - 1) * HPC + 1, :],
                                 e[:, TILE_N : 2 * TILE_N], start=False,
                                 stop=True)
                for h, pv in ((0, pv0), (1, pv1)):
                    dn = small.tile([1, TILE_N], FP32, tag="dn")
                    nc.vector.tensor_copy(dn[:], pv[D : D + 1, :])
                    rd = small.tile([1, TILE_N], FP32, tag="rd")
                    nc.vector.reciprocal_approx_fast(rd[:], dn[:])
                    bc = small.tile([D, TILE_N], FP32, tag="bc")
                    nc.gpsimd.partition_broadcast(bc[:], rd[:])
                    nc.vector.tensor_mul(
                        ctxT[h * D : (h + 1) * D, isl], pv[0:D, :], bc[:])

            # drain leftover fillers for this batch
            pull(n_fill)

        # tail: output projection for the last batch
        for _ in gen_wo(B - 1):
            pass


_CACHE = {}


def _get_nc():
    if "nc" not in _CACHE:
        nc = bacc.Bacc("TRN2", target_bir_lowering=False, debug=False,
                       num_devices=NCORES)
        with tile.TileContext(nc) as tc:
            build_kernel_body(tc)
        nc.compile()
        _CACHE["nc"] = nc
    return _CACHE["nc"]


def host_prep(x, Wq, bq, Wk, bk, Wv, bv, Wo, bo):
    import ml_dtypes
    bf16 = ml_dtypes.bfloat16
    x = np.asarray(x, np.float32)
    xT = np.ascontiguousarray(x.reshape(TT, C).T.astype(bf16))
    scale = np.float32(1.0 / np.sqrt(D))
    in_maps = []
    for c in range(NCORES):
        fsl = slice(c * F, (c + 1) * F)
        in_maps.append({
            "xT": xT,
            "wq": np.ascontiguousarray(
                (np.asarray(Wq, np.float32)[:, fsl] * scale).astype(bf16)),
            "wk": np.ascontiguousarray(np.asarray(Wk, np.float32)[:, fsl].astype(bf16)),
            "wv": np.ascontiguousarray(np.asarray(Wv, np.float32)[:, fsl].astype(bf16)),
            "wo": np.ascontiguousarray(np.asarray(Wo, np.float32)[fsl, :].astype(bf16)),
            "bq": np.ascontiguousarray(np.asarray(bq, np.float32)[fsl] * scale),
            "bk": np.ascontiguousarray(np.asarray(bk, np.float32)[fsl]),
        })
    return in_maps


def host_gather(results, Wo, bo, bv):
    total = np.zeros((C, TT), np.float64)
    for c in range(NCORES):
        total += results[c]["outT"].astype(np.float64)
    out = total.T.astype(np.float32)
    out = out + (np.asarray(bo, np.float32)
                 + np.asarray(bv, np.float32) @ np.asarray(Wo, np.float32))
    return out.reshape(B, T, C)


def _install_profile_hook():
    """Make trace=True work under axon when antenv.axon_hooks is absent."""
    import sys
    import types

    try:
        import antenv.axon_hooks  # noqa: F401
        return
    except ImportError:
        pass
    import antenv
    from trn_agent_boot.trn_boot import _ntff_profile_via_ctypes

    mod = types.ModuleType("antenv.axon_hooks")
    holder = [None]
    mod.set_axon_ntff_profile_hook = lambda h: holder.__setitem__(0, h)
    mod.get_axon_ntff_profile_hook = lambda: holder[0]
    sys.modules["antenv.axon_hooks"] = mod
    antenv.axon_hooks = mod
    mod.set_axon_ntff_profile_hook(
        _ntff_profile_via_ctypes("/opt/axon/libaxon_pjrt.so")
    )
    # artifact upload needs internal storage; keep profiles local
    import concourse.bass_utils as bu
    bu.upload_artifacts = lambda tmpdir: f"local:{tmpdir}"


def kernel(x, Wq, bq, Wk, bk, Wv, bv, Wo, bo, _trace=False):
    if _trace:
        _install_profile_hook()
    nc = _get_nc()
    in_maps = host_prep(x, Wq, bq, Wk, bk, Wv, bv, Wo, bo)
    res = run_bass_kernel_spmd(nc, in_maps, core_ids=list(range(NCORES)),
                               trace=_trace)
    _CACHE["last_result"] = res
    return host_gather(res.results, Wo, bo, bv)


# revision 13
# speedup vs baseline: 1.0632x; 1.0632x over previous
"""Multi-head self-attention (B=4, T=2048, C=1024, H=16, D=64) on 8 NeuronCores.

Sharding: tensor-parallel over heads (Megatron): each core owns 2 heads.
Wq/Wk/Wv column-sharded, Wo row-sharded; host sums the 8 partial outputs.

Device layout is fully "transposed" (features on partitions, tokens on the
free dim) so that softmax runs over the PSUM free dim and the PV matmul needs
no attention-matrix transpose. The softmax denominator comes from a ones
column appended to V (M=65 stationary), landing in row 64 of the PV PSUM.

The PE executes its stream in order, so projection/output-projection matmuls
of adjacent batches are interleaved into the attention j-loop (which is
paced by the Scalar engine's exp) to keep both engines near 100% busy.
"""

import numpy as np

import concourse.bass as bass
import concourse.tile as tile
from concourse import bacc, mybir
from concourse.bass_utils import run_bass_kernel_spmd

B, T, C, H, D = 4, 2048, 1024, 16, 64
NCORES = 8
HPC = H // NCORES          # heads per core = 2
F = HPC * D                # per-core feature width = 128
TT = B * T                 # total tokens = 8192

FP32 = mybir.dt.float32
MM_DT = mybir.dt.bfloat16  # matmul compute dtype

TILE_K = 128               # contraction tile
TILE_N = 512               # moving free dim per matmul
NK_C = C // TILE_K         # 8 k-tiles over channels
NT4 = T // TILE_N          # 4 token chunks per batch
NJ = T // TILE_K           # 16 key tiles per batch


def build_kernel_body(tc):
    nc = tc.nc
    Exp = mybir.ActivationFunctionType.Exp

    xT = nc.dram_tensor("xT", [C, TT], MM_DT, kind="ExternalInput").ap()
    wq = nc.dram_tensor("wq", [C, F], MM_DT, kind="ExternalInput").ap()
    wk = nc.dram_tensor("wk", [C, F], MM_DT, kind="ExternalInput").ap()
    wv = nc.dram_tensor("wv", [C, F], MM_DT, kind="ExternalInput").ap()
    wo = nc.dram_tensor("wo", [F, C], MM_DT, kind="ExternalInput").ap()
    bqv = nc.dram_tensor("bq", [F], FP32, kind="ExternalInput").ap()
    bkv = nc.dram_tensor("bk", [F], FP32, kind="ExternalInput").ap()
    outT = nc.dram_tensor("outT", [C, TT], FP32, kind="ExternalOutput").ap()

    import contextlib
    ctx = contextlib.ExitStack()
    with ctx:
        consts = ctx.enter_context(tc.tile_pool(name="consts", bufs=1))
        xpool = ctx.enter_context(tc.tile_pool(name="xt", bufs=12))
        bigs = ctx.enter_context(tc.tile_pool(name="bigs", bufs=2))
        epool = ctx.enter_context(tc.tile_pool(name="expp", bufs=4))
        small = ctx.enter_context(tc.tile_pool(name="small", bufs=4))
        vstage = ctx.enter_context(tc.tile_pool(name="vstage", bufs=2))
        ps_qk = ctx.enter_context(tc.tile_pool(name="ps_qk", bufs=2, space="PSUM"))
        ps_pv = ctx.enter_context(tc.tile_pool(name="ps_pv", bufs=1, space="PSUM"))
        ps_aux = ctx.enter_context(tc.tile_pool(name="ps_aux", bufs=2, space="PSUM"))

        # ---- constants ----
        wq_sb = consts.tile([TILE_K, C], MM_DT)  # c-tile k at [:, k*F:(k+1)*F]
        nc.sync.dma_start(
            wq_sb[:].rearrange("p (k f) -> p k f", k=NK_C),
            wq.rearrange("(k p) f -> p k f", p=TILE_K))
        wk_sb = consts.tile([TILE_K, C], MM_DT)
        nc.sync.dma_start(
            wk_sb[:].rearrange("p (k f) -> p k f", k=NK_C),
            wk.rearrange("(k p) f -> p k f", p=TILE_K))
        wv_sb = consts.tile([TILE_K, C], MM_DT)
        nc.sync.dma_start(
            wv_sb[:].rearrange("p (k f) -> p k f", k=NK_C),
            wv.rearrange("(k p) f -> p k f", p=TILE_K))
        wo_sb = consts.tile([F, C], MM_DT)
        nc.sync.dma_start(wo_sb[:], wo)
        bq_sb = consts.tile([F, 1], FP32)
        nc.sync.dma_start(bq_sb[:], bqv.rearrange("(p one) -> p one", one=1))
        bk_sb = consts.tile([F, 1], FP32)
        nc.sync.dma_start(bk_sb[:], bkv.rearrange("(p one) -> p one", one=1))
        ident = consts.tile([128, 128], FP32)
        from concourse.masks import make_identity
        make_identity(nc, ident[:])
        ones32 = consts.tile([128, NJ * HPC], FP32)
        nc.gpsimd.memset(ones32[:], 1.0)

        tiles = {}  # per-batch SBUF tiles

        def alloc_proj_tiles(b):
            qT = bigs.tile([F, T], MM_DT, tag="qT", name=f"qT{b}")
            kT = bigs.tile([F, T], MM_DT, tag="kT", name=f"kT{b}")
            v1 = bigs.tile([128, NJ * HPC, D + 1], MM_DT, tag="v1",
                           name=f"v1_{b}")
            nc.vector.tensor_copy(
                v1[:, :, D : D + 1],
                ones32[:].rearrange("p (a b) -> p a b", b=1),
            )
            tiles[b] = {"qT": qT, "kT": kT, "v1": v1}

        def gen_proj(b):
            """Generator: projections for batch b; yields after each PE op."""
            alloc_proj_tiles(b)
            tl = tiles[b]
            t0 = b * T
            for t4 in range(NT4):
                xts = []
                for kk in range(NK_C):
                    xt = xpool.tile([TILE_K, TILE_N], MM_DT, tag="xt")
                    nc.sync.dma_start(
                        xt[:],
                        xT[kk * TILE_K : (kk + 1) * TILE_K,
                           t0 + t4 * TILE_N : t0 + (t4 + 1) * TILE_N],
                    )
                    xts.append(xt)
                for which, w_sb in (("q", wq_sb), ("k", wk_sb), ("v", wv_sb)):
                    acc = ps_aux.tile([128, TILE_N], FP32, tag="aux")
                    for kk in range(NK_C):
                        nc.tensor.matmul(
                            acc[:], w_sb[:, kk * F : (kk + 1) * F], xts[kk][:],
                            start=(kk == 0), stop=(kk == NK_C - 1),
                        )
                        yield
                    if which == "q":
                        nc.vector.tensor_scalar_add(
                            tl["qT"][:, t4 * TILE_N : (t4 + 1) * TILE_N],
                            acc[:], bq_sb[:])
                    elif which == "k":
                        nc.vector.tensor_scalar_add(
                            tl["kT"][:, t4 * TILE_N : (t4 + 1) * TILE_N],
                            acc[:], bk_sb[:])
                    else:
                        vt_sb = vstage.tile([128, TILE_N], FP32, tag="vt")
                        nc.any.tensor_copy(vt_sb[:], acc[:])
                        for tt in range(TILE_N // 128):
                            j_idx = t4 * (TILE_N // 128) + tt
                            ptr = ps_aux.tile([128, TILE_N], FP32, tag="aux")
                            nc.tensor.transpose(
                                ptr[:, 0:128],
                                vt_sb[:, tt * 128 : (tt + 1) * 128], ident[:],
                            )
                            # both heads' v columns in one strided copy
                            nc.vector.tensor_copy(
                                tl["v1"][:, j_idx * HPC : (j_idx + 1) * HPC, 0:D],
                                ptr[:, 0:128].rearrange("p (h d) -> p h d", h=HPC),
                            )
                            yield

        def gen_wo_t4(b, t4):
            """Generator: output projection chunk; yields per PE op."""
            t0 = b * T
            ctxT = tiles[b]["ctxT"]
            if True:
                for o in range(C // 128):
                    po = ps_aux.tile([128, TILE_N], FP32, tag="aux")
                    nc.tensor.matmul(
                        po[:], wo_sb[:, o * 128 : (o + 1) * 128],
                        ctxT[:, t4 * TILE_N : (t4 + 1) * TILE_N],
                        start=True, stop=True,
                    )
                    osb = vstage.tile([128, TILE_N], FP32, tag="osb", bufs=4)
                    nc.any.tensor_copy(osb[:], po[:])
                    nc.sync.dma_start(
                        outT[o * 128 : (o + 1) * 128,
                             t0 + t4 * TILE_N : t0 + (t4 + 1) * TILE_N],
                        osb[:],
                    )
                    yield

        fillers = []

        def pull(budget):
            while budget > 0 and fillers:
                try:
                    next(fillers[0])
                    budget -= 1
                except StopIteration:
                    fillers.pop(0)

        # prologue: projections for batch 0 (PE-only ramp)
        for _ in gen_proj(0):
            pass

        for b in range(B):
            tl = tiles[b]
            ctxT = bigs.tile([F, T], MM_DT, tag="ctxT", name=f"ctxT{b}")
            tl["ctxT"] = ctxT
            qT, kT, v1 = tl["qT"], tl["kT"], tl["v1"]

            if b + 1 < B:
                fillers.append(gen_proj(b + 1))

            for i4 in range(NT4):
                isl = slice(i4 * TILE_N, (i4 + 1) * TILE_N)
                pv0 = ps_pv.tile([128, TILE_N], FP32, tag="pv0")
                pv1 = ps_pv.tile([128, TILE_N], FP32, tag="pv1")
                expts = [None] * NJ
                for j in range(NJ):
                    jsl = slice(j * TILE_K, (j + 1) * TILE_K)
                    qk = ps_qk.tile([128, 2 * TILE_N], FP32, tag="qk")
                    # heads in distinct PE row-groups -> run concurrently
                    nc.tensor.matmul(qk[:, 0:TILE_N], kT[0:D, jsl],
                                     qT[0:D, isl], start=True, stop=True)
                    nc.tensor.matmul(qk[:, TILE_N : 2 * TILE_N],
                                     kT[D : 2 * D, jsl], qT[D : 2 * D, isl],
                                     start=True, stop=True)
                    expt = epool.tile([128, 2 * TILE_N], MM_DT, tag="expt")
                    nc.scalar.activation(expt[:], qk[:], Exp)
                    expts[j] = expt
                    # keep the in-order PE stream fed while ACT runs exp
                    pull(2)
                    if j > 0:
                        e = expts[j - 1]
                        nc.tensor.matmul(
                            pv0[0 : D + 1, :], v1[:, (j - 1) * HPC + 0, :],
                            e[:, 0:TILE_N], start=(j == 1), stop=False)
                        nc.tensor.matmul(
                            pv1[0 : D + 1, :], v1[:, (j - 1) * HPC + 1, :],
                            e[:, TILE_N : 2 * TILE_N], start=(j == 1),
                            stop=False)
                e = expts[NJ - 1]
                nc.tensor.matmul(pv0[0 : D + 1, :], v1[:, (NJ - 1) * HPC + 0, :],
                                 e[:, 0:TILE_N], start=False, stop=True)
                nc.tensor.matmul(pv1[0 : D + 1, :], v1[:, (NJ - 1) * HPC + 1, :],
                                 e[:, TILE_N : 2 * TILE_N], start=False,
                                 stop=True)
                for h, pv in ((0, pv0), (1, pv1)):
                    dn = small.tile([1, TILE_N], FP32, tag="dn")
                    nc.vector.tensor_copy(dn[:], pv[D : D + 1, :])
                    rd = small.tile([1, TILE_N], FP32, tag="rd")
                    nc.vector.reciprocal_approx_fast(rd[:], dn[:])
                    bc = small.tile([D, TILE_N], FP32, tag="bc")
                    nc.gpsimd.partition_broadcast(bc[:], rd[:])
                    nc.vector.tensor_mul(
                        ctxT[h * D : (h + 1) * D, isl], pv[0:D, :], bc[:])
                fillers.append(gen_wo_t4(b, i4))

        # drain remaining fillers (last batch's final wo chunks)
        pull(10 ** 9)


_CACHE = {}


def _get_nc():
    if "nc" not in _CACHE:
        nc = bacc.Bacc("TRN2", target_bir_lowering=False, debug=False,
                       num_devices=NCORES)
        with tile.TileContext(nc) as tc:
            build_kernel_body(tc)
        nc.compile()
        _CACHE["nc"] = nc
    return _CACHE["nc"]


def host_prep(x, Wq, bq, Wk, bk, Wv, bv, Wo, bo):
    import ml_dtypes
    bf16 = ml_dtypes.bfloat16
    x = np.asarray(x, np.float32)
    xT = np.ascontiguousarray(x.reshape(TT, C).T.astype(bf16))
    scale = np.float32(1.0 / np.sqrt(D))
    in_maps = []
    for c in range(NCORES):
        fsl = slice(c * F, (c + 1) * F)
        in_maps.append({
            "xT": xT,
            "wq": np.ascontiguousarray(
                (np.asarray(Wq, np.float32)[:, fsl] * scale).astype(bf16)),
            "wk": np.ascontiguousarray(np.asarray(Wk, np.float32)[:, fsl].astype(bf16)),
            "wv": np.ascontiguousarray(np.asarray(Wv, np.float32)[:, fsl].astype(bf16)),
            "wo": np.ascontiguousarray(np.asarray(Wo, np.float32)[fsl, :].astype(bf16)),
            "bq": np.ascontiguousarray(np.asarray(bq, np.float32)[fsl] * scale),
            "bk": np.ascontiguousarray(np.asarray(bk, np.float32)[fsl]),
        })
    return in_maps


def host_gather(results, Wo, bo, bv):
    total = np.zeros((C, TT), np.float64)
    for c in range(NCORES):
        total += results[c]["outT"].astype(np.float64)
    out = total.T.astype(np.float32)
    out = out + (np.asarray(bo, np.float32)
                 + np.asarray(bv, np.float32) @ np.asarray(Wo, np.float32))
    return out.reshape(B, T, C)


def _install_profile_hook():
    """Make trace=True work under axon when antenv.axon_hooks is absent."""
    import sys
    import types

    try:
        import antenv.axon_hooks  # noqa: F401
        return
    except ImportError:
        pass
    import antenv
    from trn_agent_boot.trn_boot import _ntff_profile_via_ctypes

    mod = types.ModuleType("antenv.axon_hooks")
    holder = [None]
    mod.set_axon_ntff_profile_hook = lambda h: holder.__setitem__(0, h)
    mod.get_axon_ntff_profile_hook = lambda: holder[0]
    sys.modules["antenv.axon_hooks"] = mod
    antenv.axon_hooks = mod
    mod.set_axon_ntff_profile_hook(
        _ntff_profile_via_ctypes("/opt/axon/libaxon_pjrt.so")
    )
    # artifact upload needs internal storage; keep profiles local
    import concourse.bass_utils as bu
    bu.upload_artifacts = lambda tmpdir: f"local:{tmpdir}"


def kernel(x, Wq, bq, Wk, bk, Wv, bv, Wo, bo, _trace=False):
    if _trace:
        _install_profile_hook()
    nc = _get_nc()
    in_maps = host_prep(x, Wq, bq, Wk, bk, Wv, bv, Wo, bo)
    res = run_bass_kernel_spmd(nc, in_maps, core_ids=list(range(NCORES)),
                               trace=_trace)
    _CACHE["last_result"] = res
    return host_gather(res.results, Wo, bo, bv)


# revision 14
# speedup vs baseline: 1.2503x; 1.1760x over previous
"""Multi-head self-attention (B=4, T=2048, C=1024, H=16, D=64) on 8 NeuronCores.

Sharding: tensor-parallel over heads (Megatron): each core owns 2 heads.
Wq/Wk/Wv column-sharded, Wo row-sharded; host sums the 8 partial outputs.

Device layout is fully "transposed" (features on partitions, tokens on the
free dim) so that softmax runs over the PSUM free dim and the PV matmul needs
no attention-matrix transpose. The softmax denominator comes from a ones
column appended to V (M=65 stationary), landing in row 64 of the PV PSUM.

The PE executes its stream in order, so projection/output-projection matmuls
of adjacent batches are interleaved into the attention j-loop (which is
paced by the Scalar engine's exp) to keep both engines near 100% busy.
"""

import numpy as np

import concourse.bass as bass
import concourse.tile as tile
from concourse import bacc, mybir
from concourse.bass_utils import run_bass_kernel_spmd

B, T, C, H, D = 4, 2048, 1024, 16, 64
NCORES = 8
HPC = H // NCORES          # heads per core = 2
F = HPC * D                # per-core feature width = 128
TT = B * T                 # total tokens = 8192

FP32 = mybir.dt.float32
MM_DT = mybir.dt.bfloat16  # matmul compute dtype

TILE_K = 128               # contraction tile
TILE_N = 512               # moving free dim per matmul
NK_C = C // TILE_K         # 8 k-tiles over channels
NT4 = T // TILE_N          # 4 token chunks per batch
NJ = T // TILE_K           # 16 key tiles per batch


def build_kernel_body(tc):
    nc = tc.nc
    Exp = mybir.ActivationFunctionType.Exp

    xT = nc.dram_tensor("xT", [C, TT], MM_DT, kind="ExternalInput").ap()
    wq = nc.dram_tensor("wq", [C, F], MM_DT, kind="ExternalInput").ap()
    wk = nc.dram_tensor("wk", [C, F], MM_DT, kind="ExternalInput").ap()
    wv = nc.dram_tensor("wv", [C, F], MM_DT, kind="ExternalInput").ap()
    wo = nc.dram_tensor("wo", [F, C], MM_DT, kind="ExternalInput").ap()
    bqv = nc.dram_tensor("bq", [F], FP32, kind="ExternalInput").ap()
    bkv = nc.dram_tensor("bk", [F], FP32, kind="ExternalInput").ap()
    outT = nc.dram_tensor("outT", [C, TT], FP32, kind="ExternalOutput").ap()

    import contextlib
    ctx = contextlib.ExitStack()
    with ctx:
        consts = ctx.enter_context(tc.tile_pool(name="consts", bufs=1))
        xpool = ctx.enter_context(tc.tile_pool(name="xt", bufs=12))
        bigs = ctx.enter_context(tc.tile_pool(name="bigs", bufs=2))
        epool = ctx.enter_context(tc.tile_pool(name="expp", bufs=4))
        small = ctx.enter_context(tc.tile_pool(name="small", bufs=4))
        vstage = ctx.enter_context(tc.tile_pool(name="vstage", bufs=2))
        ps_qk = ctx.enter_context(tc.tile_pool(name="ps_qk", bufs=2, space="PSUM"))
        ps_pv = ctx.enter_context(tc.tile_pool(name="ps_pv", bufs=1, space="PSUM"))
        ps_aux = ctx.enter_context(tc.tile_pool(name="ps_aux", bufs=2, space="PSUM"))

        # ---- constants ----
        wq_sb = consts.tile([TILE_K, C], MM_DT)  # c-tile k at [:, k*F:(k+1)*F]
        nc.sync.dma_start(
            wq_sb[:].rearrange("p (k f) -> p k f", k=NK_C),
            wq.rearrange("(k p) f -> p k f", p=TILE_K))
        wk_sb = consts.tile([TILE_K, C], MM_DT)
        nc.sync.dma_start(
            wk_sb[:].rearrange("p (k f) -> p k f", k=NK_C),
            wk.rearrange("(k p) f -> p k f", p=TILE_K))
        wv_sb = consts.tile([TILE_K, C], MM_DT)
        nc.sync.dma_start(
            wv_sb[:].rearrange("p (k f) -> p k f", k=NK_C),
            wv.rearrange("(k p) f -> p k f", p=TILE_K))
        wo_sb = consts.tile([F, C], MM_DT)
        nc.sync.dma_start(wo_sb[:], wo)
        bq_sb = consts.tile([F, 1], FP32)
        nc.sync.dma_start(bq_sb[:], bqv.rearrange("(p one) -> p one", one=1))
        bk_sb = consts.tile([F, 1], FP32)
        nc.sync.dma_start(bk_sb[:], bkv.rearrange("(p one) -> p one", one=1))
        ident = consts.tile([128, 128], FP32)
        from concourse.masks import make_identity
        make_identity(nc, ident[:])
        ones32 = consts.tile([128, NJ * HPC], FP32)
        nc.gpsimd.memset(ones32[:], 1.0)

        tiles = {}  # per-batch SBUF tiles

        def alloc_proj_tiles(b):
            qT = bigs.tile([F, T], MM_DT, tag="qT", name=f"qT{b}")
            kT = bigs.tile([F, T], MM_DT, tag="kT", name=f"kT{b}")
            v1 = bigs.tile([128, NJ * HPC, D + 1], MM_DT, tag="v1",
                           name=f"v1_{b}")
            nc.vector.tensor_copy(
                v1[:, :, D : D + 1],
                ones32[:].rearrange("p (a b) -> p a b", b=1),
            )
            tiles[b] = {"qT": qT, "kT": kT, "v1": v1}

        def gen_proj(b):
            """Generator: projections for batch b; yields after each PE op."""
            alloc_proj_tiles(b)
            tl = tiles[b]
            t0 = b * T
            for t4 in range(NT4):
                xts = []
                for kk in range(NK_C):
                    xt = xpool.tile([TILE_K, TILE_N], MM_DT, tag="xt")
                    nc.sync.dma_start(
                        xt[:],
                        xT[kk * TILE_K : (kk + 1) * TILE_K,
                           t0 + t4 * TILE_N : t0 + (t4 + 1) * TILE_N],
                    )
                    xts.append(xt)
                for which, w_sb in (("q", wq_sb), ("k", wk_sb), ("v", wv_sb)):
                    acc = ps_aux.tile([128, TILE_N], FP32, tag="aux")
                    for kk in range(NK_C):
                        nc.tensor.matmul(
                            acc[:], w_sb[:, kk * F : (kk + 1) * F], xts[kk][:],
                            start=(kk == 0), stop=(kk == NK_C - 1),
                        )
                        yield
                    if which == "q":
                        nc.vector.tensor_scalar_add(
                            tl["qT"][:, t4 * TILE_N : (t4 + 1) * TILE_N],
                            acc[:], bq_sb[:])
                    elif which == "k":
                        nc.vector.tensor_scalar_add(
                            tl["kT"][:, t4 * TILE_N : (t4 + 1) * TILE_N],
                            acc[:], bk_sb[:])
                    else:
                        vt_sb = vstage.tile([128, TILE_N], FP32, tag="vt")
                        nc.vector.tensor_copy(vt_sb[:], acc[:])
                        for tt in range(TILE_N // 128):
                            j_idx = t4 * (TILE_N // 128) + tt
                            ptr = ps_aux.tile([128, TILE_N], FP32, tag="aux")
                            nc.tensor.transpose(
                                ptr[:, 0:128],
                                vt_sb[:, tt * 128 : (tt + 1) * 128], ident[:],
                            )
                            # both heads' v columns in one strided copy
                            nc.vector.tensor_copy(
                                tl["v1"][:, j_idx * HPC : (j_idx + 1) * HPC, 0:D],
                                ptr[:, 0:128].rearrange("p (h d) -> p h d", h=HPC),
                            )
                            yield

        def gen_wo_t4(b, t4):
            """Generator: output projection chunk; yields per PE op."""
            t0 = b * T
            ctxT = tiles[b]["ctxT"]
            if True:
                for o in range(C // 128):
                    po = ps_aux.tile([128, TILE_N], FP32, tag="aux")
                    nc.tensor.matmul(
                        po[:], wo_sb[:, o * 128 : (o + 1) * 128],
                        ctxT[:, t4 * TILE_N : (t4 + 1) * TILE_N],
                        start=True, stop=True,
                    )
                    osb = vstage.tile([128, TILE_N], FP32, tag="osb", bufs=4)
                    nc.vector.tensor_copy(osb[:], po[:])
                    nc.sync.dma_start(
                        outT[o * 128 : (o + 1) * 128,
                             t0 + t4 * TILE_N : t0 + (t4 + 1) * TILE_N],
                        osb[:],
                    )
                    yield

        fillers = []

        def pull(budget):
            while budget > 0 and fillers:
                try:
                    next(fillers[0])
                    budget -= 1
                except StopIteration:
                    fillers.pop(0)

        # prologue: projections for batch 0 (PE-only ramp)
        for _ in gen_proj(0):
            pass

        for b in range(B):
            tl = tiles[b]
            ctxT = bigs.tile([F, T], MM_DT, tag="ctxT", name=f"ctxT{b}")
            tl["ctxT"] = ctxT
            qT, kT, v1 = tl["qT"], tl["kT"], tl["v1"]

            if b + 1 < B:
                fillers.append(gen_proj(b + 1))

            for i4 in range(NT4):
                isl = slice(i4 * TILE_N, (i4 + 1) * TILE_N)
                pv0 = ps_pv.tile([128, TILE_N], FP32, tag="pv0")
                pv1 = ps_pv.tile([128, TILE_N], FP32, tag="pv1")
                expts = [None] * NJ
                for j in range(NJ):
                    jsl = slice(j * TILE_K, (j + 1) * TILE_K)
                    qk = ps_qk.tile([128, 2 * TILE_N], FP32, tag="qk")
                    # heads in distinct PE row-groups -> run concurrently
                    nc.tensor.matmul(qk[:, 0:TILE_N], kT[0:D, jsl],
                                     qT[0:D, isl], start=True, stop=True)
                    nc.tensor.matmul(qk[:, TILE_N : 2 * TILE_N],
                                     kT[D : 2 * D, jsl], qT[D : 2 * D, isl],
                                     start=True, stop=True)
                    expt = epool.tile([128, 2 * TILE_N], MM_DT, tag="expt")
                    nc.scalar.activation(expt[:], qk[:], Exp)
                    expts[j] = expt
                    # keep the in-order PE stream fed while ACT runs exp
                    pull(2)
                    if j >= 2:
                        jj = j - 2
                        e = expts[jj]
                        nc.tensor.matmul(
                            pv0[0 : D + 1, :], v1[:, jj * HPC + 0, :],
                            e[:, 0:TILE_N], start=(jj == 0), stop=False)
                        nc.tensor.matmul(
                            pv1[0 : D + 1, :], v1[:, jj * HPC + 1, :],
                            e[:, TILE_N : 2 * TILE_N], start=(jj == 0),
                            stop=False)
                for jj in (NJ - 2, NJ - 1):
                    e = expts[jj]
                    nc.tensor.matmul(pv0[0 : D + 1, :], v1[:, jj * HPC + 0, :],
                                     e[:, 0:TILE_N], start=False,
                                     stop=(jj == NJ - 1))
                    nc.tensor.matmul(pv1[0 : D + 1, :], v1[:, jj * HPC + 1, :],
                                     e[:, TILE_N : 2 * TILE_N], start=False,
                                     stop=(jj == NJ - 1))
                for h, pv in ((0, pv0), (1, pv1)):
                    dn = small.tile([1, TILE_N], FP32, tag="dn")
                    nc.vector.tensor_copy(dn[:], pv[D : D + 1, :])
                    rd = small.tile([1, TILE_N], FP32, tag="rd")
                    nc.vector.reciprocal_approx_fast(rd[:], dn[:])
                    bc = small.tile([D, TILE_N], FP32, tag="bc")
                    nc.gpsimd.partition_broadcast(bc[:], rd[:])
                    nc.vector.tensor_mul(
                        ctxT[h * D : (h + 1) * D, isl], pv[0:D, :], bc[:])
                fillers.append(gen_wo_t4(b, i4))

        # drain remaining fillers (last batch's final wo chunks)
        pull(10 ** 9)


_CACHE = {}


def _get_nc():
    if "nc" not in _CACHE:
        nc = bacc.Bacc("TRN2", target_bir_lowering=False, debug=False,
                       num_devices=NCORES)
        with tile.TileContext(nc) as tc:
            build_kernel_body(tc)
        nc.compile()
        _CACHE["nc"] = nc
    return _CACHE["nc"]


def host_prep(x, Wq, bq, Wk, bk, Wv, bv, Wo, bo):
    import ml_dtypes
    bf16 = ml_dtypes.bfloat16
    x = np.asarray(x, np.float32)
    xT = np.ascontiguousarray(x.reshape(TT, C).T.astype(bf16))
    scale = np.float32(1.0 / np.sqrt(D))
    in_maps = []
    for c in range(NCORES):
        fsl = slice(c * F, (c + 1) * F)
        in_maps.append({
            "xT": xT,
            "wq": np.ascontiguousarray(
                (np.asarray(Wq, np.float32)[:, fsl] * scale).astype(bf16)),
            "wk": np.ascontiguousarray(np.asarray(Wk, np.float32)[:, fsl].astype(bf16)),
            "wv": np.ascontiguousarray(np.asarray(Wv, np.float32)[:, fsl].astype(bf16)),
            "wo": np.ascontiguousarray(np.asarray(Wo, np.float32)[fsl, :].astype(bf16)),
            "bq": np.ascontiguousarray(np.asarray(bq, np.float32)[fsl] * scale),
            "bk": np.ascontiguousarray(np.asarray(bk, np.float32)[fsl]),
        })
    return in_maps


def host_gather(results, Wo, bo, bv):
    total = np.zeros((C, TT), np.float64)
    for c in range(NCORES):
        total += results[c]["outT"].astype(np.float64)
    out = total.T.astype(np.float32)
    out = out + (np.asarray(bo, np.float32)
                 + np.asarray(bv, np.float32) @ np.asarray(Wo, np.float32))
    return out.reshape(B, T, C)


def _install_profile_hook():
    """Make trace=True work under axon when antenv.axon_hooks is absent."""
    import sys
    import types

    try:
        import antenv.axon_hooks  # noqa: F401
        return
    except ImportError:
        pass
    import antenv
    from trn_agent_boot.trn_boot import _ntff_profile_via_ctypes

    mod = types.ModuleType("antenv.axon_hooks")
    holder = [None]
    mod.set_axon_ntff_profile_hook = lambda h: holder.__setitem__(0, h)
    mod.get_axon_ntff_profile_hook = lambda: holder[0]
    sys.modules["antenv.axon_hooks"] = mod
    antenv.axon_hooks = mod
    mod.set_axon_ntff_profile_hook(
        _ntff_profile_via_ctypes("/opt/axon/libaxon_pjrt.so")
    )
    # artifact upload needs internal storage; keep profiles local
    import concourse.bass_utils as bu
    bu.upload_artifacts = lambda tmpdir: f"local:{tmpdir}"


def kernel(x, Wq, bq, Wk, bk, Wv, bv, Wo, bo, _trace=False):
    if _trace:
        _install_profile_hook()
    nc = _get_nc()
    in_maps = host_prep(x, Wq, bq, Wk, bk, Wv, bv, Wo, bo)
    res = run_bass_kernel_spmd(nc, in_maps, core_ids=list(range(NCORES)),
                               trace=_trace)
    _CACHE["last_result"] = res
    return host_gather(res.results, Wo, bo, bv)
